# revision 1
# baseline (speedup 1.0000x reference)
"""Self-contained Trainium2 Bass kernel for the GCN encoder layer
(GCNConv + PReLU), distributed over 8 NeuronCores.

    out = PReLU(A_hat @ x @ W + b),  A_hat = D^-1/2 (A + I) D^-1/2

Strategy (host does partitioning/indexing only; all FLOPs on device):
  * Destination nodes are sharded across the 8 cores (12500 rows each).
  * Because aggregation is linear, we aggregate in x-space first
    (agg = A_hat @ x), then apply the dense transform (agg @ W) — this
    avoids materializing and re-gathering h = x @ W.
  * Per core, destinations are packed into "bins" of 128 (bin membership
    is chosen by a balancing packer, not by node id); each bin owns a
    static number of 128-edge "tiles" (slots). Every incoming edge
    (including self loops) occupies one slot of its destination's bin.
  * Edge source rows are fetched with dma_gather. Its indices are int16,
    so bins are grouped into 3 segments; each segment gets its own
    compacted source-row table (the unique sources referenced by that
    segment's edges, < 32768 rows), uploaded per core.
  * Per tile, a scaled one-hot matrix S[e, j] = (j == dstoff_e) * norm_e
    is built with one DVE tensor_scalar (iota compare), and the segment
    sum is computed on the tensor engine: aggT[c, d] += Xg^T @ S,
    accumulated in PSUM over the bin's tiles.
  * Bin epilogue: out2[o, d] = W^T @ aggT (W stationary), then
    PReLU(out2 + b) built from two Relu activations (per-partition bias,
    channels on partitions) and one DVE scalar_tensor_tensor.
  * The kernel writes out_t [128 ch, 12544 dst] per core (transposed,
    bin-permuted); the host transposes back and un-permutes.
"""

import numpy as np

import concourse.bass as bass
import concourse.bacc as bacc
import concourse.tile as tile
import concourse.mybir as mybir
from concourse.bass_utils import run_bass_kernel_spmd

F32 = mybir.dt.float32
I16 = mybir.dt.int16

N = 100000
C = 128
P = 128
NCORES = 8
PER = N // NCORES            # 12500
NBINS = (PER + P - 1) // P   # 98
DPAD = NBINS * P             # 12544
NSEG = 3
WIN = 32768

CH_TILES = 8                 # dma_gather is limited to 1024 idx per call
META_TILES = 64
N_QUEUES = 4
XG_BUFS = 4                  # must be <= N_QUEUES (one in-flight gather
                             # per SWDGE queue ring of 1024 descriptors)


# ----------------------------------------------------------------------
# host-side preprocessing
# ----------------------------------------------------------------------

def _pack_core_capped(deg_local, caps):
    """Assign PER dsts (+ pads) to NBINS bins of exactly P dsts with
    per-bin edge capacity caps. Feasibility-aware balancing greedy."""
    npad = DPAD - len(deg_local)
    deg_all = np.concatenate([deg_local,
                              np.zeros(npad, dtype=deg_local.dtype)])
    order = np.argsort(-deg_all, kind="stable")
    slots_left = np.full(NBINS, P, dtype=np.int64)
    cap_left = np.asarray(caps, dtype=np.int64).copy()
    load = np.zeros(NBINS, dtype=np.int64)
    bin_of = np.empty(DPAD, dtype=np.int64)
    pos_of = np.empty(DPAD, dtype=np.int64)
    for dd in order:
        dg = deg_all[dd]
        ok = (slots_left > 0) & (cap_left >= dg)
        if not ok.any():
            return None
        cand = np.where(ok)[0]
        bb = cand[np.argmax(cap_left[cand] / slots_left[cand])]
        bin_of[dd] = bb
        pos_of[dd] = P - slots_left[bb]
        slots_left[bb] -= 1
        cap_left[bb] -= dg
        load[bb] += dg
    return bin_of, pos_of, load


def _build_all(src, dst):
    deg = np.bincount(dst, minlength=N).astype(np.int64) + 1
    dis = 1.0 / np.sqrt(deg.astype(np.float64))
    core_of = dst // PER

    k8s = []
    for c in range(NCORES):
        deg_c = deg[c * PER:(c + 1) * PER]
        edges_c = int(deg_c.sum())
        k8s.append(max(0, -(-(edges_c - NBINS * 7 * P) // P)) + 2)
    K8 = max(k8s)
    packs = None
    while packs is None and K8 <= NBINS:
        caps = np.array([8 * P] * K8 + [7 * P] * (NBINS - K8), dtype=np.int64)
        packs = []
        for c in range(NCORES):
            r = _pack_core_capped(deg[c * PER:(c + 1) * PER], caps)
            if r is None:
                packs = None
                K8 += 2
                break
            packs.append(r)
    assert packs is not None, "bin packing failed"
    tiles_of_bin = np.array([8] * K8 + [7] * (NBINS - K8), dtype=np.int64)
    G = int(tiles_of_bin.sum())
    tile_base = np.concatenate([[0], np.cumsum(tiles_of_bin)])[:-1]

    cum = np.cumsum(tiles_of_bin)
    bin_bounds = [0]
    for s in range(1, NSEG):
        bin_bounds.append(int(np.searchsorted(cum, G * s / NSEG)))
    bin_bounds.append(NBINS)
    seg_tile_bounds = [0] + [int(tile_base[b]) if b < NBINS else G
                             for b in bin_bounds[1:]]
    seg_of_bin = np.zeros(NBINS, dtype=np.int64)
    for s in range(NSEG):
        seg_of_bin[bin_bounds[s]:bin_bounds[s + 1]] = s

    per_core_edges = []
    for c in range(NCORES):
        mask = core_of == c
        e_src = src[mask]
        e_dstl = dst[mask] - c * PER
        all_src = np.concatenate(
            [e_src, np.arange(c * PER, (c + 1) * PER, dtype=np.int64)])
        all_dstl = np.concatenate([e_dstl, np.arange(PER, dtype=np.int64)])
        per_core_edges.append((all_src, all_dstl))

    seg_uniq = np.zeros((NCORES, NSEG), dtype=np.int64)
    core_sorted = []
    for c in range(NCORES):
        bin_of, pos_of, load = packs[c]
        all_src, all_dstl = per_core_edges[c]
        b_of_e = bin_of[all_dstl]
        order = np.argsort(b_of_e, kind="stable")
        s_sorted = all_src[order]
        b_sorted = b_of_e[order]
        seg_sorted = seg_of_bin[b_sorted]
        lists = []
        for s in range(NSEG):
            ss = s_sorted[seg_sorted == s]
            _, first_pos = np.unique(ss, return_index=True)
            uniq = ss[np.sort(first_pos)]
            assert len(uniq) < WIN, (c, s, len(uniq))
            seg_uniq[c, s] = len(uniq)
            lists.append(uniq)
        core_sorted.append((order, s_sorted, b_sorted, seg_sorted, lists))

    maxu = seg_uniq.max(axis=0)
    seg_bases = np.concatenate([[0], np.cumsum(maxu)])[:-1].astype(np.int64)
    xc_rows = int(seg_bases[-1] + WIN)

    static = dict(tiles_of_bin=tiles_of_bin, tile_base=tile_base, G=G,
                  seg_tile_bounds=seg_tile_bounds, seg_bases=seg_bases,
                  xc_rows=xc_rows, seg_of_bin=seg_of_bin)

    cores = []
    for c in range(NCORES):
        bin_of, pos_of, load = packs[c]
        all_src, all_dstl = per_core_edges[c]
        order, s_sorted, b_sorted, seg_sorted, lists = core_sorted[c]

        lid = np.empty(len(s_sorted), dtype=np.int64)
        xc_rowsrc = np.full(xc_rows, -1, dtype=np.int64)
        for s in range(NSEG):
            m = seg_sorted == s
            uniq = lists[s]
            lmap = np.full(N, -1, dtype=np.int64)
            lmap[uniq] = np.arange(len(uniq))
            lid[m] = lmap[s_sorted[m]]
            xc_rowsrc[seg_bases[s]:seg_bases[s] + len(uniq)] = uniq

        counts = np.bincount(b_sorted, minlength=NBINS)
        run_start = np.concatenate([[0], np.cumsum(counts)])[:-1]
        within = np.arange(len(b_sorted)) - run_start[b_sorted]
        g = tile_base[b_sorted] + within // P
        p = within % P
        slot = g * P + p

        srcidx_local = np.zeros(G * P, dtype=np.int64)
        normv = np.zeros(G * P, dtype=np.float32)
        dstoffv = np.zeros(G * P, dtype=np.float32)
        srcidx_local[slot] = lid
        all_dst_global = all_dstl[order] + c * PER
        normv[slot] = (dis[s_sorted] * dis[all_dst_global]).astype(np.float32)
        dstoffv[slot] = pos_of[all_dstl[order]].astype(np.float32)

        idx16 = np.zeros((16, G * 8), dtype=np.int16)
        i = np.arange(G * P)
        idx16[i % 16, i // 16] = srcidx_local.astype(np.int16)
        idx16 = np.tile(idx16, (8, 1))

        norm = normv.reshape(G, P).T.copy()
        dstoff = dstoffv.reshape(G, P).T.copy()
        outrow_of_dst = bin_of * P + pos_of
        cores.append(dict(xc_rowsrc=xc_rowsrc, idx16=idx16, norm=norm,
                          dstoff=dstoff, outrow_of_dst=outrow_of_dst))
    return static, cores


# ----------------------------------------------------------------------
# device program
# ----------------------------------------------------------------------

def _build_program(static, repeat=1):
    tiles_of_bin = static["tiles_of_bin"]
    seg_tile_bounds = static["seg_tile_bounds"]
    seg_bases = static["seg_bases"]
    xc_rows = static["xc_rows"]
    G = int(np.sum(tiles_of_bin))

    nc = bacc.Bacc("TRN2", target_bir_lowering=False, debug=False,
                   num_devices=NCORES, num_swdge_queues=N_QUEUES,
                   dynamic_dma_scratch_size=65536)

    xc_d = nc.dram_tensor("xc", [xc_rows, C], F32, kind="ExternalInput")
    ix_d = nc.dram_tensor("idx16", [P, 8 * G], I16, kind="ExternalInput")
    nm_d = nc.dram_tensor("edgenorm", [P, G], F32, kind="ExternalInput")
    do_d = nc.dram_tensor("dstoff", [P, G], F32, kind="ExternalInput")
    w_d = nc.dram_tensor("Wt", [C, C], F32, kind="ExternalInput")
    bias_d = nc.dram_tensor("bias", [C, 1], F32, kind="ExternalInput")
    nbias_d = nc.dram_tensor("nbias", [C, 1], F32, kind="ExternalInput")
    nalpha_d = nc.dram_tensor("nalpha", [C, 1], F32, kind="ExternalInput")
    iota_d = nc.dram_tensor("iota", [P, P], F32, kind="ExternalInput")
    out_d = nc.dram_tensor("out_t", [C, DPAD], F32, kind="ExternalOutput")

    chunks = []
    nseg = len(seg_tile_bounds) - 1
    for s in range(nseg):
        t0, t1 = seg_tile_bounds[s], seg_tile_bounds[s + 1]
        g = t0
        while g < t1:
            e = min(g + CH_TILES, t1)
            chunks.append((g, e, s))
            g = e
    chunk_of_tile = {}
    for ci, (g0, g1, s) in enumerate(chunks):
        for g in range(g0, g1):
            chunk_of_tile[g] = ci

    mbatches = []
    cur_m0 = None
    for (g0, g1, s) in chunks:
        if cur_m0 is None:
            cur_m0 = g0
        if g1 - cur_m0 > META_TILES:
            mbatches.append((cur_m0, g0))
            cur_m0 = g0
    mbatches.append((cur_m0, G))
    mb_of_tile = {}
    for mi, (m0, m1) in enumerate(mbatches):
        for g in range(m0, m1):
            mb_of_tile[g] = mi

    with tile.TileContext(nc) as tc:
        with (
            tc.tile_pool(name="const", bufs=1) as constp,
            tc.tile_pool(name="xg", bufs=XG_BUFS) as xgp,
            tc.tile_pool(name="meta", bufs=3) as metap,
            tc.tile_pool(name="s", bufs=6) as sp,
            tc.tile_pool(name="aggts", bufs=4) as aggp,
            tc.tile_pool(name="res", bufs=6) as resp,
            tc.tile_pool(name="psA", bufs=4, space="PSUM") as psA,
            tc.tile_pool(name="psB", bufs=4, space="PSUM") as psB,
        ):
            w_sb = constp.tile([C, C], F32)
            iota_sb = constp.tile([P, P], F32)
            b_sb = constp.tile([C, 1], F32)
            nb_sb = constp.tile([C, 1], F32)
            na_sb = constp.tile([C, 1], F32)
            nc.sync.dma_start(out=w_sb[:], in_=w_d[:, :])
            nc.sync.dma_start(out=iota_sb[:], in_=iota_d[:, :])
            nc.sync.dma_start(out=b_sb[:], in_=bias_d[:, :])
            nc.sync.dma_start(out=nb_sb[:], in_=nbias_d[:, :])
            nc.sync.dma_start(out=na_sb[:], in_=nalpha_d[:, :])

            cur = {}
            curm = {}
            rep_state = {"r": 0}

            def load_meta(mi):
                m0, m1 = mbatches[mi]
                M = m1 - m0
                idx = metap.tile([P, 8 * META_TILES], I16, tag="idx")
                nrm = metap.tile([P, META_TILES], F32, tag="nrm")
                dof = metap.tile([P, META_TILES], F32, tag="dof")
                nc.sync.dma_start(out=idx[:, :8 * M],
                                  in_=ix_d[:, 8 * m0:8 * m1])
                nc.sync.dma_start(out=nrm[:, :M], in_=nm_d[:, m0:m1])
                nc.sync.dma_start(out=dof[:, :M], in_=do_d[:, m0:m1])
                curm[mi] = (idx, nrm, dof, m0)

            def load_chunk(ci):
                g0, g1, s = chunks[ci]
                K = g1 - g0
                mi = mb_of_tile[g0]
                if mi not in curm:
                    load_meta(mi)
                idx, _, _, m0 = curm[mi]
                xg = xgp.tile([P, CH_TILES, C], F32, tag="xg")
                base = int(seg_bases[s])
                nc.gpsimd.dma_gather(
                    out_ap=xg[:, :K, :],
                    in_ap=xc_d[base:base + WIN, :],
                    idxs_ap=idx[:, 8 * (g0 - m0):8 * (g1 - m0)],
                    num_idxs=K * P,
                    num_idxs_reg=K * P,
                    elem_size=C,
                    queue_num=(rep_state["r"] * len(chunks) + ci) % N_QUEUES,
                )
                cur[ci] = (xg, g0)

            # repeat>1 re-runs the same body (timing aid)
            for _rep in range(repeat):
                rep_state["r"] = _rep
                cur.clear()
                curm.clear()
                g = 0
                for b, T in enumerate(tiles_of_bin):
                    aggT = psA.tile([C, P], F32, tag="aggT")
                    for t in range(T):
                        ci = chunk_of_tile[g]
                        if ci not in cur:
                            load_chunk(ci)
                        xg, g0 = cur[ci]
                        mi = mb_of_tile[g]
                        _, nrm, dof, m0 = curm[mi]
                        k = g - g0
                        km = g - m0
                        S = sp.tile([P, P], F32, tag="S")
                        nc.vector.tensor_scalar(
                            out=S[:],
                            in0=iota_sb[:],
                            scalar1=dof[:, km:km + 1],
                            scalar2=nrm[:, km:km + 1],
                            op0=mybir.AluOpType.is_equal,
                            op1=mybir.AluOpType.mult,
                        )
                        nc.tensor.matmul(
                            out=aggT[:],
                            lhsT=xg[:, k, :],
                            rhs=S[:],
                            start=(t == 0),
                            stop=(t == T - 1),
                        )
                        g += 1
                    aggTs = aggp.tile([C, P], F32, tag="aggTs")
                    nc.scalar.activation(
                        out=aggTs[:], in_=aggT[:],
                        func=mybir.ActivationFunctionType.Copy,
                    )
                    out2 = psB.tile([C, P], F32, tag="out2")
                    nc.tensor.matmul(out=out2[:], lhsT=w_sb[:], rhs=aggTs[:],
                                     start=True, stop=True)
                    pos = resp.tile([C, P], F32, tag="pos")
                    nc.scalar.activation(
                        out=pos[:], in_=out2[:],
                        func=mybir.ActivationFunctionType.Relu,
                        bias=b_sb[:, :1], scale=1.0,
                    )
                    neg = resp.tile([C, P], F32, tag="neg")
                    nc.scalar.activation(
                        out=neg[:], in_=out2[:],
                        func=mybir.ActivationFunctionType.Relu,
                        bias=nb_sb[:, :1], scale=-1.0,
                    )
                    res = resp.tile([C, P], F32, tag="res")
                    nc.vector.scalar_tensor_tensor(
                        out=res[:],
                        in0=neg[:],
                        scalar=na_sb[:, :1],
                        in1=pos[:],
                        op0=mybir.AluOpType.mult,
                        op1=mybir.AluOpType.add,
                    )
                    nc.sync.dma_start(out=out_d[:, b * P:(b + 1) * P],
                                      in_=res[:])


    nc.compile()
    return nc


# ----------------------------------------------------------------------
# public entry point
# ----------------------------------------------------------------------

_CACHE = {}


def _get_compiled(src, dst):
    key = (src.tobytes(), dst.tobytes())
    h = hash(key)
    if h not in _CACHE:
        static, cores = _build_all(src, dst)
        nc = _build_program(static)
        _CACHE[h] = (static, cores, nc)
    return _CACHE[h]


def kernel(x, edge_index, W, b, prelu_w):
    x = np.ascontiguousarray(np.asarray(x, dtype=np.float32))
    ei = np.asarray(edge_index)
    W = np.asarray(W, dtype=np.float32)
    b = np.asarray(b, dtype=np.float32)
    prelu_w = np.asarray(prelu_w, dtype=np.float32)
    src = ei[0].astype(np.int64)
    dst = ei[1].astype(np.int64)
    assert x.shape == (N, C), x.shape

    static, cores, nc = _get_compiled(src, dst)

    iota = np.tile(np.arange(P, dtype=np.float32), (P, 1))
    xc_rows = static["xc_rows"]
    in_maps = []
    for ca in cores:
        rowsrc = ca["xc_rowsrc"]
        xc = np.zeros((xc_rows, C), dtype=np.float32)
        valid = rowsrc >= 0
        xc[valid] = x[rowsrc[valid]]
        in_maps.append({
            "xc": xc,
            "idx16": ca["idx16"],
            "edgenorm": ca["norm"],
            "dstoff": ca["dstoff"],
            "Wt": W,
            "bias": b.reshape(C, 1),
            "nbias": (-b).reshape(C, 1),
            "nalpha": (-prelu_w).reshape(C, 1),
            "iota": iota,
        })

    res = None
    for attempt in range(3):
        try:
            res = run_bass_kernel_spmd(nc, in_maps,
                                       core_ids=list(range(NCORES)))
            break
        except Exception:
            if attempt == 2:
                raise
            import time as _time
            _time.sleep(20.0)

    out = np.empty((N, C), dtype=np.float32)
    for c, ca in enumerate(cores):
        ot = res.results[c]["out_t"]          # [C, DPAD]
        oc = np.ascontiguousarray(ot.T)       # [DPAD, C]
        out[c * PER:(c + 1) * PER] = oc[ca["outrow_of_dst"][:PER]]
    return out



# revision 3
# speedup vs baseline: 3.0940x; 3.0940x over previous
"""Self-contained Trainium2 Bass kernel for the GCN encoder layer
(GCNConv + PReLU), distributed over 8 NeuronCores.

    out = PReLU(A_hat @ x @ W + b),  A_hat = D^-1/2 (A + I) D^-1/2

Strategy v2 (bf16 streaming, host does partitioning/indexing/layout):
  * Destination nodes are sharded across the 8 cores (12500 rows each).
  * Aggregation is done in x-space first (agg = A_hat @ x), then the
    dense transform (agg @ W) per 512-dst group.
  * Per core, destinations are packed into bins of 128 (balancing
    packer); each bin owns 7 or 8 static 128-edge tiles. Every edge
    (incl. self loops) occupies one slot.
  * The host pre-gathers x[src] per slot into a partition-major bf16
    "image" xg[p, g*C:(g+1)*C] = x[src(g,p)], so the device streams it
    with large contiguous DMA descriptors (4KB/partition/chunk) instead
    of per-edge 512B gather descriptors. bf16 halves the traffic.
  * Per tile, a scaled one-hot S[e, j] = (j == dstoff_e) * norm_e is
    built with one DVE tensor_scalar (iota compare, all-bf16 operands
    for the fast DVE mode); the segment sum runs on the tensor engine:
    aggT[c, d] += Xg^T @ S, accumulated in PSUM. bf16 matmuls run at
    1 cycle/row vs 4 for fp32.
  * Epilogue per 4-bin group (512 dsts, one PSUM bank): one Copy
    (PSUM->SBUF, cast bf16), one W matmul, two Relu activations
    (per-partition bias) and one DVE scalar_tensor_tensor for PReLU.
  * out_t [128 ch, 12544 dst] bf16 per core; host upcasts, transposes,
    and un-permutes.
"""

import numpy as np
import ml_dtypes

import concourse.bass as bass
import concourse.bacc as bacc
import concourse.tile as tile
import concourse.mybir as mybir
from concourse.bass_utils import run_bass_kernel_spmd

F32 = mybir.dt.float32
BF16 = mybir.dt.bfloat16
NPBF16 = ml_dtypes.bfloat16

N = 100000
C = 128
P = 128
NCORES = 8
PER = N // NCORES            # 12500
NBINS = (PER + P - 1) // P   # 98
DPAD = NBINS * P             # 12544
GRP = 4                      # bins per epilogue group (512 dsts = 1 PSUM bank)
CH_TILES = 16                # tiles per DMA chunk (4KB/partition/descriptor)


# ----------------------------------------------------------------------
# host-side preprocessing
# ----------------------------------------------------------------------

def _pack_core_capped(deg_local, caps):
    """Assign PER dsts (+ pads) to NBINS bins of exactly P dsts with
    per-bin edge capacity caps. Feasibility-aware balancing greedy."""
    npad = DPAD - len(deg_local)
    deg_all = np.concatenate([deg_local,
                              np.zeros(npad, dtype=deg_local.dtype)])
    order = np.argsort(-deg_all, kind="stable")
    slots_left = np.full(NBINS, P, dtype=np.int64)
    cap_left = np.asarray(caps, dtype=np.int64).copy()
    bin_of = np.empty(DPAD, dtype=np.int64)
    pos_of = np.empty(DPAD, dtype=np.int64)
    for dd in order:
        dg = deg_all[dd]
        ok = (slots_left > 0) & (cap_left >= dg)
        if not ok.any():
            return None
        cand = np.where(ok)[0]
        bb = cand[np.argmax(cap_left[cand] / slots_left[cand])]
        bin_of[dd] = bb
        pos_of[dd] = P - slots_left[bb]
        slots_left[bb] -= 1
        cap_left[bb] -= dg
    return bin_of, pos_of


def _build_all(src, dst):
    deg = np.bincount(dst, minlength=N).astype(np.int64) + 1
    dis = 1.0 / np.sqrt(deg.astype(np.float64))
    core_of = dst // PER

    k8s = []
    for c in range(NCORES):
        deg_c = deg[c * PER:(c + 1) * PER]
        edges_c = int(deg_c.sum())
        k8s.append(max(0, -(-(edges_c - NBINS * 7 * P) // P)) + 2)
    K8 = max(k8s)
    packs = None
    while packs is None and K8 <= NBINS:
        caps = np.array([8 * P] * K8 + [7 * P] * (NBINS - K8), dtype=np.int64)
        packs = []
        for c in range(NCORES):
            r = _pack_core_capped(deg[c * PER:(c + 1) * PER], caps)
            if r is None:
                packs = None
                K8 += 2
                break
            packs.append(r)
    assert packs is not None, "bin packing failed"
    tiles_of_bin = np.array([8] * K8 + [7] * (NBINS - K8), dtype=np.int64)
    G = int(tiles_of_bin.sum())
    tile_base = np.concatenate([[0], np.cumsum(tiles_of_bin)])[:-1]

    static = dict(tiles_of_bin=tiles_of_bin, tile_base=tile_base, G=G)

    cores = []
    for c in range(NCORES):
        bin_of, pos_of = packs[c]
        mask = core_of == c
        e_src = src[mask]
        e_dstl = dst[mask] - c * PER
        all_src = np.concatenate(
            [e_src, np.arange(c * PER, (c + 1) * PER, dtype=np.int64)])
        all_dstl = np.concatenate([e_dstl, np.arange(PER, dtype=np.int64)])

        b_of_e = bin_of[all_dstl]
        order = np.argsort(b_of_e, kind="stable")
        s_sorted = all_src[order]
        b_sorted = b_of_e[order]

        counts = np.bincount(b_sorted, minlength=NBINS)
        run_start = np.concatenate([[0], np.cumsum(counts)])[:-1]
        within = np.arange(len(b_sorted)) - run_start[b_sorted]
        g = tile_base[b_sorted] + within // P
        p = within % P
        slot = g * P + p

        srcidx = np.zeros(G * P, dtype=np.int64)
        normv = np.zeros(G * P, dtype=np.float64)
        dofv = np.zeros(G * P, dtype=np.float64)
        srcidx[slot] = s_sorted
        all_dst_global = all_dstl[order] + c * PER
        normv[slot] = dis[s_sorted] * dis[all_dst_global]
        dofv[slot] = pos_of[all_dstl[order]].astype(np.float64)

        norm = normv.reshape(G, P).T.astype(np.float32).copy()
        dof = dofv.reshape(G, P).T.astype(np.float32).copy()
        outrow_of_dst = bin_of * P + pos_of
        cores.append(dict(srcidx=srcidx.reshape(G, P), norm=norm, dof=dof,
                          outrow_of_dst=outrow_of_dst))
    return static, cores


# ----------------------------------------------------------------------
# device program
# ----------------------------------------------------------------------

def _build_program(static, repeat=1):
    tiles_of_bin = static["tiles_of_bin"]
    G = static["G"]

    nc = bacc.Bacc("TRN2", target_bir_lowering=False, debug=False,
                   num_devices=NCORES)

    xg_d = nc.dram_tensor("xg", [P, G * C], BF16, kind="ExternalInput")
    nm_d = nc.dram_tensor("edgenorm", [P, G], F32, kind="ExternalInput")
    do_d = nc.dram_tensor("dstoff", [P, G], F32, kind="ExternalInput")
    w_d = nc.dram_tensor("Wt", [C, C], BF16, kind="ExternalInput")
    bias_d = nc.dram_tensor("bias", [C, 1], F32, kind="ExternalInput")
    nbias_d = nc.dram_tensor("nbias", [C, 1], F32, kind="ExternalInput")
    nalpha_d = nc.dram_tensor("nalpha", [C, 1], F32, kind="ExternalInput")
    iota_d = nc.dram_tensor("iota", [P, P], BF16, kind="ExternalInput")
    out_d = nc.dram_tensor("out_t", [C, DPAD], BF16, kind="ExternalOutput")

    groups = []
    for b0 in range(0, NBINS, GRP):
        groups.append(list(range(b0, min(b0 + GRP, NBINS))))

    with tile.TileContext(nc) as tc:
        with (
            tc.tile_pool(name="const", bufs=1) as constp,
            tc.tile_pool(name="xg", bufs=6) as xgp,
            tc.tile_pool(name="s", bufs=8) as sp,
            tc.tile_pool(name="aggts", bufs=3) as aggp,
            tc.tile_pool(name="res", bufs=6) as resp,
            tc.tile_pool(name="psA", bufs=2, space="PSUM") as psA,
            tc.tile_pool(name="psB", bufs=2, space="PSUM") as psB,
        ):
            w_sb = constp.tile([C, C], BF16)
            iota_sb = constp.tile([P, P], BF16)
            b_sb = constp.tile([C, 1], F32)
            nb_sb = constp.tile([C, 1], F32)
            na_sb = constp.tile([C, 1], F32)
            nrm_sb = constp.tile([P, G], F32)
            dof_sb = constp.tile([P, G], F32)
            nc.sync.dma_start(out=w_sb[:], in_=w_d[:, :])
            nc.sync.dma_start(out=iota_sb[:], in_=iota_d[:, :])
            nc.sync.dma_start(out=b_sb[:], in_=bias_d[:, :])
            nc.sync.dma_start(out=nb_sb[:], in_=nbias_d[:, :])
            nc.sync.dma_start(out=na_sb[:], in_=nalpha_d[:, :])
            nc.sync.dma_start(out=nrm_sb[:], in_=nm_d[:, :])
            nc.sync.dma_start(out=dof_sb[:], in_=do_d[:, :])

            cur = {}

            def load_chunk(ci):
                g0 = ci * CH_TILES
                g1 = min(g0 + CH_TILES, G)
                xg = xgp.tile([P, CH_TILES * C], BF16, tag="xg")
                nc.sync.dma_start(out=xg[:, :(g1 - g0) * C],
                                  in_=xg_d[:, g0 * C:g1 * C])
                cur[ci] = (xg, g0)

            for _rep in range(repeat):
                cur.clear()
                g = 0
                for group in groups:
                    gw = len(group)
                    agg = psA.tile([C, GRP * P], F32, tag="agg")
                    for j, b in enumerate(group):
                        T = int(tiles_of_bin[b])
                        for t in range(T):
                            ci = g // CH_TILES
                            if ci not in cur:
                                load_chunk(ci)
                            xg, g0 = cur[ci]
                            k = g - g0
                            S = sp.tile([P, P], BF16, tag="S")
                            nc.vector.tensor_scalar(
                                out=S[:],
                                in0=iota_sb[:],
                                scalar1=dof_sb[:, g:g + 1],
                                scalar2=nrm_sb[:, g:g + 1],
                                op0=mybir.AluOpType.is_equal,
                                op1=mybir.AluOpType.mult,
                            )
                            nc.tensor.matmul(
                                out=agg[:, j * P:(j + 1) * P],
                                lhsT=xg[:, k * C:(k + 1) * C],
                                rhs=S[:],
                                start=(t == 0),
                                stop=(t == T - 1),
                            )
                            g += 1
                    aggTs = aggp.tile([C, GRP * P], BF16, tag="aggTs")
                    nc.scalar.activation(
                        out=aggTs[:, :gw * P], in_=agg[:, :gw * P],
                        func=mybir.ActivationFunctionType.Copy,
                    )
                    out2 = psB.tile([C, GRP * P], F32, tag="out2")
                    nc.tensor.matmul(out=out2[:, :gw * P], lhsT=w_sb[:],
                                     rhs=aggTs[:, :gw * P],
                                     start=True, stop=True)
                    pos = resp.tile([C, GRP * P], BF16, tag="pos")
                    nc.scalar.activation(
                        out=pos[:, :gw * P], in_=out2[:, :gw * P],
                        func=mybir.ActivationFunctionType.Relu,
                        bias=b_sb[:, :1], scale=1.0,
                    )
                    neg = resp.tile([C, GRP * P], BF16, tag="neg")
                    nc.scalar.activation(
                        out=neg[:, :gw * P], in_=out2[:, :gw * P],
                        func=mybir.ActivationFunctionType.Relu,
                        bias=nb_sb[:, :1], scale=-1.0,
                    )
                    res = resp.tile([C, GRP * P], BF16, tag="res")
                    nc.vector.scalar_tensor_tensor(
                        out=res[:, :gw * P],
                        in0=neg[:, :gw * P],
                        scalar=na_sb[:, :1],
                        in1=pos[:, :gw * P],
                        op0=mybir.AluOpType.mult,
                        op1=mybir.AluOpType.add,
                    )
                    b0 = group[0]
                    nc.sync.dma_start(
                        out=out_d[:, b0 * P:b0 * P + gw * P],
                        in_=res[:, :gw * P])

    nc.compile()
    return nc


# ----------------------------------------------------------------------
# public entry point
# ----------------------------------------------------------------------

_CACHE = {}


def _get_compiled(src, dst):
    key = hash((src.tobytes(), dst.tobytes()))
    if key not in _CACHE:
        static, cores = _build_all(src, dst)
        nc = _build_program(static)
        _CACHE[key] = (static, cores, nc)
    return _CACHE[key]


def _make_in_maps(static, cores, x, W, b, prelu_w):
    G = static["G"]
    x_bf = x.astype(NPBF16)
    iota = np.tile(np.arange(P, dtype=NPBF16), (P, 1))
    in_maps = []
    for ca in cores:
        # partition-major image: xg_img[p, g*C:(g+1)*C] = x[src(g, p)]
        xg = x_bf[ca["srcidx"]]                     # [G, P, C]
        xg_img = np.ascontiguousarray(
            xg.transpose(1, 0, 2)).reshape(P, G * C)
        in_maps.append({
            "xg": xg_img,
            "edgenorm": ca["norm"],
            "dstoff": ca["dof"],
            "Wt": W.astype(NPBF16),
            "bias": b.reshape(C, 1).astype(np.float32),
            "nbias": (-b).reshape(C, 1).astype(np.float32),
            "nalpha": (-prelu_w).reshape(C, 1).astype(np.float32),
            "iota": iota,
        })
    return in_maps


def kernel(x, edge_index, W, b, prelu_w):
    x = np.ascontiguousarray(np.asarray(x, dtype=np.float32))
    ei = np.asarray(edge_index)
    W = np.asarray(W, dtype=np.float32)
    b = np.asarray(b, dtype=np.float32)
    prelu_w = np.asarray(prelu_w, dtype=np.float32)
    src = ei[0].astype(np.int64)
    dst = ei[1].astype(np.int64)
    assert x.shape == (N, C), x.shape

    static, cores, nc = _get_compiled(src, dst)
    in_maps = _make_in_maps(static, cores, x, W, b, prelu_w)

    res = None
    for attempt in range(3):
        try:
            res = run_bass_kernel_spmd(nc, in_maps,
                                       core_ids=list(range(NCORES)))
            break
        except Exception:
            if attempt == 2:
                raise
            import time as _time
            _time.sleep(20.0)

    out = np.empty((N, C), dtype=np.float32)
    for c, ca in enumerate(cores):
        ot = res.results[c]["out_t"].astype(np.float32)   # [C, DPAD]
        oc = np.ascontiguousarray(ot.T)                   # [DPAD, C]
        out[c * PER:(c + 1) * PER] = oc[ca["outrow_of_dst"][:PER]]
    return out
